# revision 14
# baseline (speedup 1.0000x reference)
"""AnisotropicEdgeFilter Trainium2 kernel (8 NeuronCores, data-parallel over edges).

Math (per edge e):
    h  = elu(pos @ W1 + b1)                       [E, 128]
    ew = (h @ W2 + b2).reshape(E, 8, 32)          per-edge filter
    out[e, o] = sum_i attr[e, i] * ew[e, i, o]    [E, 32]

Device-side restructuring (v2):
    elu(x) = relu(x) - relu(1 - exp(x))           (exact identity)
      a = Relu(x)            [ScalarE, PSUM->SBUF]
      e = Exp(x)             [ScalarE, PSUM->SBUF]
      b = Relu(-e + 1)       [ScalarE, SBUF->SBUF, via activation pre-affine]
    ew = a @ W2 + b @ (-W2)  -- the subtraction happens inside the PE's PSUM
      accumulation (two accumulating matmuls per 128-edge subtile), so no
      elementwise combine op is ever needed for h.
    einsum: prod[p,s,i,o] = ew[p,s,i,o] * attr[p,s,i] (VectorE, one op per
      512 edges, contiguous innermost dim), then the i-reduction as a 3-step
      halving tree of tensor_tensor adds (8->4->2->1), each with 32-wide
      contiguous runs so the DVE 2x bf16 mode applies.
    b1 is folded via a ones-row augmentation of pos/W1 (exact).
    b2 is zero in this problem; a nonzero b2 is handled exactly by a host-side
      epilogue add (attr @ b2.reshape(8,32)).

Layouts: x/a/b/e in [hidden=partition, edge=free]; einsum in
[edge=partition, (subtile, i, o)=free].
"""

import sys

import numpy as np

sys.path.insert(0, "/opt/trn_rl_repo")

import ml_dtypes  # noqa: E402

E = 500000
IN_SIZE = 8
POS_SIZE = 6
HIDDEN = 128
OUT_SIZE = 32
N_CORES = 8
CHUNK = 2048                  # edges per chunk (16 subtiles of 128)
N_CHUNKS = 31
SUBT = CHUNK // 128           # 16 subtiles per chunk
E_LOC = CHUNK * N_CHUNKS      # 63488 edges per core
E_PAD = E_LOC * N_CORES       # 507904

_BF16 = ml_dtypes.bfloat16

_COMPILED = {}


def _build_nc():
    import concourse.bass as bass  # noqa: F401
    import concourse.tile as tile
    from concourse import bacc, mybir

    dt = mybir.dt
    nc = bacc.Bacc(
        "TRN2",
        target_bir_lowering=False,
        debug=False,
        num_devices=N_CORES,
    )

    post_d = nc.dram_tensor("post", [POS_SIZE + 1, E_LOC], dt.bfloat16, kind="ExternalInput")
    attr_d = nc.dram_tensor("attr", [128, N_CHUNKS, SUBT, IN_SIZE], dt.bfloat16, kind="ExternalInput")
    w1_d = nc.dram_tensor("w1aug", [POS_SIZE + 1, HIDDEN], dt.bfloat16, kind="ExternalInput")
    w2_d = nc.dram_tensor("w2", [HIDDEN, IN_SIZE * OUT_SIZE], dt.bfloat16, kind="ExternalInput")
    w2n_d = nc.dram_tensor("w2n", [HIDDEN, IN_SIZE * OUT_SIZE], dt.bfloat16, kind="ExternalInput")
    out_d = nc.dram_tensor("out", [N_CHUNKS, 128, SUBT, OUT_SIZE], dt.bfloat16, kind="ExternalOutput")

    ACT = mybir.ActivationFunctionType
    ALU = mybir.AluOpType

    with tile.TileContext(nc) as tc:
        with (
            tc.tile_pool(name="wpool", bufs=1) as wpool,
            tc.tile_pool(name="pospool", bufs=3) as pospool,
            tc.tile_pool(name="hps", bufs=1, space="PSUM") as hps_pool,
            tc.tile_pool(name="ewps", bufs=2, space="PSUM") as ewps_pool,
            tc.tile_pool(name="actp", bufs=2) as actp,
            tc.tile_pool(name="work", bufs=3) as work,
            tc.tile_pool(name="outp", bufs=3) as outp,
        ):
            post_ap = post_d.ap()
            out_ap = out_d.ap()
            attr_ap = attr_d.ap()

            # startup order: w1 + pos0 first so mm1 can begin ASAP; attr
            # pieces go on the scalar HWDGE queue to issue in parallel.
            w1_sb = wpool.tile([POS_SIZE + 1, HIDDEN], dt.bfloat16)
            nc.sync.dma_start(w1_sb[:], w1_d.ap())
            pos0_sb = pospool.tile([POS_SIZE + 1, CHUNK], dt.bfloat16, tag="pos")
            nc.sync.dma_start(pos0_sb[:], post_ap[:, 0:CHUNK])
            w2_sb = wpool.tile([HIDDEN, IN_SIZE * OUT_SIZE], dt.bfloat16)
            nc.sync.dma_start(w2_sb[:], w2_d.ap())
            w2n_sb = wpool.tile([HIDDEN, IN_SIZE * OUT_SIZE], dt.bfloat16)
            nc.sync.dma_start(w2n_sb[:], w2n_d.ap())
            # all of attr stays resident: 7.9 KiB/partition
            attr_sb = wpool.tile([128, N_CHUNKS, SUBT, IN_SIZE], dt.bfloat16)
            nc.scalar.dma_start(attr_sb[:, 0:2], attr_ap[:, 0:2])
            nc.scalar.dma_start(attr_sb[:, 2:N_CHUNKS], attr_ap[:, 2:N_CHUNKS])

            def chunk_front(c, pos_sb):
                """mm1 + activations for one 2048-edge chunk."""
                if pos_sb is None:
                    pos_sb = pospool.tile([POS_SIZE + 1, CHUNK], dt.bfloat16, tag="pos")
                    nc.sync.dma_start(pos_sb[:], post_ap[:, c * CHUNK : (c + 1) * CHUNK])
                # x = W1aug.T @ pos_aug -> [hidden=128, CHUNK] in PSUM (f32)
                hps = hps_pool.tile([HIDDEN, CHUNK], dt.float32)
                for m in range(CHUNK // 512):
                    nc.tensor.matmul(
                        hps[:, m * 512 : (m + 1) * 512],
                        w1_sb[:],
                        pos_sb[:, m * 512 : (m + 1) * 512],
                        start=True,
                        stop=True,
                    )
                # elu(x) = relu(x) - relu(1 - exp(x)); subtraction folded into
                # the PSUM accumulation of the two W2 matmuls below.
                e_sb = actp.tile([HIDDEN, CHUNK], dt.bfloat16, tag="e")
                a_sb = actp.tile([HIDDEN, CHUNK], dt.bfloat16, tag="a")
                b_sb = actp.tile([HIDDEN, CHUNK], dt.bfloat16, tag="b")
                nc.scalar.activation(e_sb[:], hps[:], ACT.Exp)
                nc.scalar.activation(a_sb[:], hps[:], ACT.Relu)
                nc.scalar.activation(b_sb[:], e_sb[:], ACT.Relu, bias=1.0, scale=-1.0)
                return a_sb, b_sb

            def chunk_einsum(c, a_sb, b_sb, prod, sbase):
                """ew matmuls + attr multiply for one chunk; prod[:, sbase:sbase+16]."""
                for q in range(SUBT // 4):  # 512-edge ew batches (4 subtiles)
                    ewp = ewps_pool.tile([128, 4 * IN_SIZE * OUT_SIZE], dt.float32)
                    for t in range(4):
                        col = q * 512 + t * 128
                        sl = ewp[:, t * 256 : (t + 1) * 256]
                        nc.tensor.matmul(
                            sl, a_sb[:, col : col + 128], w2_sb[:],
                            start=True, stop=False,
                        )
                        nc.tensor.matmul(
                            sl, b_sb[:, col : col + 128], w2n_sb[:],
                            start=False, stop=True,
                        )
                    s0 = q * 4
                    # prod[p, s, i, o] = ew[p, s, i, o] * attr[p, c, s0+s, i]
                    ew_v = ewp[:].rearrange(
                        "p (s i o) -> p s i o", s=4, i=IN_SIZE, o=OUT_SIZE
                    )
                    at_v = (
                        attr_sb[:, c, s0 : s0 + 4, :]
                        .unsqueeze(3)
                        .broadcast_to([128, 4, IN_SIZE, OUT_SIZE])
                    )
                    nc.vector.tensor_tensor(
                        prod[:, sbase + s0 : sbase + s0 + 4, :, :], ew_v, at_v,
                        op=ALU.mult,
                    )

            def fold_and_store(cs, prod, ns):
                """i-reduction halving tree over ns subtiles + store."""
                outt = outp.tile([128, ns, OUT_SIZE], dt.bfloat16, tag="outt")
                f1 = work.tile([128, ns, 4, OUT_SIZE], dt.bfloat16, tag="f1")
                nc.vector.tensor_tensor(
                    f1[:], prod[:, 0:ns, 0:4, :], prod[:, 0:ns, 4:8, :], op=ALU.add
                )
                f2 = work.tile([128, ns, 2, OUT_SIZE], dt.bfloat16, tag="f2")
                nc.vector.tensor_tensor(
                    f2[:], f1[:, :, 0:2, :], f1[:, :, 2:4, :], op=ALU.add
                )
                nc.vector.tensor_tensor(
                    outt[:], f2[:, :, 0, :], f2[:, :, 1, :], op=ALU.add
                )
                for k, c in enumerate(cs):
                    nc.sync.dma_start(
                        out_ap[c], outt[:, k * SUBT : (k + 1) * SUBT, :]
                    )

            pos_sb = pos0_sb
            for c in range(N_CHUNKS):
                prod = work.tile(
                    [128, SUBT, IN_SIZE, OUT_SIZE], dt.bfloat16, tag="prod"
                )
                a_sb, b_sb = chunk_front(c, pos_sb)
                pos_sb = None
                chunk_einsum(c, a_sb, b_sb, prod, 0)
                fold_and_store([c], prod, SUBT)

    nc.compile()
    return nc


def _get_compiled():
    if "nc" not in _COMPILED:
        _COMPILED["nc"] = _build_nc()
    return _COMPILED["nc"]


def _prep_shards(edge_attr, edge_pos, W1, b1, W2, b2):
    """Host-side prep: pad, fold b1, transpose, tile, cast to bf16."""
    ea = np.asarray(edge_attr, dtype=np.float32)
    ep = np.asarray(edge_pos, dtype=np.float32)
    W1 = np.asarray(W1, dtype=np.float32)
    b1 = np.asarray(b1, dtype=np.float32)
    W2 = np.asarray(W2, dtype=np.float32)

    n = ea.shape[0]
    pad = E_PAD - n
    ea_p = np.pad(ea, ((0, pad), (0, 0)))
    ep_p = np.pad(ep, ((0, pad), (0, 0)))

    w1aug = np.concatenate([W1, b1[None, :]], axis=0).astype(_BF16)  # [7, 128]
    w2_bf = W2.astype(_BF16)
    w2n_bf = (-W2).astype(_BF16)

    in_maps = []
    for i in range(N_CORES):
        sl = slice(i * E_LOC, (i + 1) * E_LOC)
        pos_sh = ep_p[sl]  # [E_LOC, 6]
        post = np.empty((POS_SIZE + 1, E_LOC), dtype=_BF16)
        post[:POS_SIZE] = pos_sh.T.astype(_BF16)
        post[POS_SIZE] = _BF16(1.0)
        # attr[p, c, s, i] with edge = c*CHUNK + s*128 + p
        attr_sh = (
            ea_p[sl].reshape(N_CHUNKS, SUBT, 128, IN_SIZE).transpose(2, 0, 1, 3)
        ).astype(_BF16)  # [128, C, S, 8]
        in_maps.append(
            {
                "post": np.ascontiguousarray(post),
                "attr": np.ascontiguousarray(attr_sh),
                "w1aug": w1aug,
                "w2": w2_bf,
                "w2n": w2n_bf,
            }
        )
    return in_maps


def kernel(**inputs) -> np.ndarray:
    from concourse.bass_utils import run_bass_kernel_spmd

    n = inputs["edge_attr"].shape[0]
    in_maps = _prep_shards(
        inputs["edge_attr"], inputs["edge_pos"],
        inputs["W1"], inputs["b1"], inputs["W2"], inputs["b2"],
    )
    nc = _get_compiled()
    res = run_bass_kernel_spmd(nc, in_maps, core_ids=list(range(N_CORES)))
    outs = []
    for i in range(N_CORES):
        o = np.asarray(res.results[i]["out"])  # [C, 128, S, 32] bf16
        o = o.astype(np.float32).transpose(0, 2, 1, 3).reshape(E_LOC, OUT_SIZE)
        outs.append(o)
    full = np.concatenate(outs, axis=0)[:n]
    b2 = np.asarray(inputs["b2"], dtype=np.float32)
    if np.any(b2):
        # exact epilogue for nonzero b2 (zero in this problem's inputs)
        full = full + np.asarray(inputs["edge_attr"], np.float32) @ b2.reshape(
            IN_SIZE, OUT_SIZE
        )
    return np.ascontiguousarray(full)
